# revision 39
# baseline (speedup 1.0000x reference)
"""ConvTranspose2d (16,256,32,32) -> (16,128,66,66), stride 2, 4x4 kernel.

Strategy: data-parallel over batch, 2 images per core on 8 NeuronCores.

Math: y[b,co,2m+p,2n+q] = bias[co]
        + sum_{i,j in {0,1}} sum_ci x[b,ci,m-i,n-j] * w[ci,co,p+2i,q+2j]
for parity class (p,q) in {0,1}^2, m,n in [0,33).

All-bf16 datapath (x, w, y; fp32 PSUM/bias): the 2e-2 rel-err gate
leaves huge headroom (measured ~3e-3), halves DMA, and avoids the
fp32-HIGH PE power mode.

Per image and parity class: output subgrid [128co x 33 x 33] as 3
row-chunks of 11 rows; each chunk is one PSUM group of 8 matmuls
(2 ci-chunks x 4 taps), K=128, M=128co, N=363 via a 2D strided rhs
over the zero-padded 34x34 SBUF x copy (padded host-side).  Drains
fuse the bias add, write contiguous bf16 class tiles; host does the
parity de-interleave.  Output leaves per class; the last class leaves
as 3 per-chunk DMAs so only ~90KB trails the final matmul.

Timing model (from traces): every [128-part, *] DMA is >=128 packets
served by the 16 shared DMA engines, so the first matmul is gated by
total packets in flight in the startup window.  Wave 1 is only
w[c0,class00] (1KB rows) + x image0 c0.  Image-0 runs tap-major
(c-outer), pushing the x0c1/w-rest deadlines to matmul 13/25.  The
image-1 x loads are held back by a WAW gate (a 1-column write into
their tiles dependent on the first class-00 drain) so their 256
packets stay out of the startup window.  The PE clock ramps until
~7us of accumulated busy time (warmups bridge the DMA window), HAM
then grants full speed for a fixed ~31-34us window, and a fixed
~257-instruction semaphore-clear teardown (~3-6us) follows the last
output DMA -- the schedule squeezes most work inside the full-speed
window.
"""

import numpy as np
import ml_dtypes

import concourse.bass as bass
import concourse.bacc as bacc
import concourse.tile as tile
from concourse import mybir
from concourse.bass_utils import run_bass_kernel_spmd

N_CORES = 8
B_PER = 2  # images per core

F32 = mybir.dt.float32
BF16 = mybir.dt.bfloat16

PW = 34            # padded x width (32 + 1 left + 1 right)
XLEN = PW * PW     # 1156 padded x elems per partition
XPAD = 1160        # sbuf/dram x free size (tail slack, keeps 4B align)
R = 11             # output parity rows per PSUM chunk
NCH = 3            # chunks: 3 * 11 = 33 parity rows
NW = 33            # useful output cols per parity row
NF = R * NW        # 363 matmul free dim (2D strided rhs AP)
NWARM = 6          # PE clock-ramp warmup matmuls: fill the PE-idle DMA
                   # window (~7.6-10us) so HAM's accumulated-busy
                   # full-speed grant comes earlier

CLASSES = [(0, 0), (0, 1), (1, 0), (1, 1)]
COPY = mybir.ActivationFunctionType.Copy
IDENT = mybir.ActivationFunctionType.Identity


def _mm(nc, ps, wt, xv, p, q, c, i, j, row0, rows, start, stop):
    r0 = row0 - i + 1
    c0 = 1 - j
    nc.tensor.matmul(
        ps[:],
        wt[c][:, 2 * p + q, (2 * i + j) * 128:(2 * i + j + 1) * 128],
        xv[c][:, r0:r0 + rows, c0:c0 + NW],
        start=start,
        stop=stop,
        skip_group_check=True,
    )


def _emit_class_tapmajor(nc, pss, wt, xv, p, q):
    """Taps outer (c0 first: x0c1 not needed until matmul 13), chunks
    inner; the 3 PSUM groups accumulate interleaved."""
    k = 0
    for c in range(2):
        for i in range(2):
            for j in range(2):
                for r in range(NCH):
                    _mm(nc, pss[r], wt, xv, p, q, c, i, j, R * r, R,
                        start=(k == 0), stop=(k == 7))
                k += 1


def _emit_group(nc, ps, wt, xv, p, q, row0, rows):
    """Chunk-major: one PSUM group of 8 matmuls (early drains)."""
    k = 0
    for c in range(2):
        for i in range(2):
            for j in range(2):
                _mm(nc, ps, wt, xv, p, q, c, i, j, row0, rows,
                    start=(k == 0), stop=(k == 7))
                k += 1


def build_nc(debug: bool = False) -> bass.Bass:
    nc = bacc.Bacc("TRN2", target_bir_lowering=False, debug=debug,
                   num_devices=N_CORES)

    # x arrives host-padded bf16: 34x34 zero-border layout + tail pad
    x_d = nc.declare_dram_parameter("x", [B_PER, 256, XPAD], BF16,
                                    isOutput=False)
    # w layout: [ci_chunk, ci, class(2p+q), tap(2i+j)*co] -- contiguous
    # per-(c,class) DRAM runs
    w_d = nc.declare_dram_parameter("w", [2, 128, 4, 512], BF16,
                                    isOutput=False)
    b_d = nc.declare_dram_parameter("b", [128, 1], F32, isOutput=False)
    # class-major output: host de-interleaves parity grids
    y_d = nc.declare_dram_parameter("y", [B_PER, 2, 2, 128, 33, NW],
                                    BF16, isOutput=True)

    with tile.TileContext(nc) as tc:
        with (
            tc.tile_pool(name="wp", bufs=2) as wpool,
            tc.tile_pool(name="bp", bufs=1) as bpool,
            tc.tile_pool(name="xp", bufs=2 * B_PER) as xpool,
            tc.tile_pool(name="cp", bufs=3) as cpool,
            tc.tile_pool(name="ps", bufs=6, space="PSUM") as ppool,
            tc.tile_pool(name="pw", bufs=1, space="PSUM") as warmpool,
        ):
            # --- tiles ----------------------------------------------
            wub = bpool.tile([128, 512], BF16)
            bt = bpool.tile([128, 1], F32)
            wt = [wpool.tile([128, 4, 512], BF16, name=f"wt{c}", tag="wt")
                  for c in range(2)]
            xp = [[xpool.tile([128, XPAD], BF16, name=f"x{b}c{c}", tag="xt")
                   for c in range(2)] for b in range(B_PER)]
            xv = [[xp[b][c][:, 0:XLEN].rearrange("p (h w) -> p h w", w=PW)
                   for c in range(2)] for b in range(B_PER)]

            # --- warmup ---------------------------------------------
            # gpsimd's program loads first: the wub memset goes there
            # so the PE can start at ~7.5us.
            nc.gpsimd.memset(wub[:], 0.0)
            wps = warmpool.tile([128, 512], F32)
            for _ in range(NWARM):
                nc.tensor.matmul(wps[:], wub[:, 0:128], wub[:],
                                 start=True, stop=True)

            # --- input DMA waves, sequenced by consumption deadline --
            # The 16 DMA engines round-robin all active rings, so
            # concurrent transfers finish together; WAW gates (1-column
            # writes into each stage's dst tiles) serialize the stages
            # so each deadline is met:
            #   wave 1 (256 pkts): w00c0 + x0c0   -> matmul 1
            #   stage A (256):     x0c1 + w1[cls00] -> matmul 13
            #   stage B (384):     w0rest + w1rest + bias -> mm 25/37
            with tc.high_priority():
                nc.sync.dma_start(out=wt[0][:, 0], in_=w_d[0, :, 0])
                nc.scalar.dma_start(out=xp[0][0][:], in_=x_d[0, 0:128])
            g = wt[0][:, 0, 0:1]
            nc.scalar.activation(xp[0][1][:, 0:1], g, COPY)
            nc.scalar.activation(wt[1][:, 0, 0:1], g, COPY)
            nc.scalar.dma_start(out=xp[0][1][:], in_=x_d[0, 128:256])
            nc.sync.dma_start(out=wt[1][:, 0], in_=w_d[1, :, 0])
            g2 = xp[0][1][:, 4:5]
            nc.scalar.activation(wt[0][:, 1, 0:1], g2, COPY)
            nc.scalar.activation(wt[1][:, 1, 0:1], g2, COPY)
            nc.scalar.activation(bt[:], g2, COPY)
            nc.sync.dma_start(out=wt[0][:, 1:4], in_=w_d[0, :, 1:4])
            nc.sync.dma_start(out=wt[1][:, 1:4], in_=w_d[1, :, 1:4])
            nc.sync.dma_start(out=bt[:], in_=b_d[:])

            # --- drains: alternate vector/scalar (gpsimd cannot read
            # PSUM) ---------------------------------------------------
            def drain(k, ct, ps, row0, rows):
                in_ = ps[:].rearrange("p (m n) -> p m n", n=NW)
                out = ct[:, row0:row0 + rows]
                if k == 1:
                    nc.scalar.activation(out, in_, IDENT, bias=bt[:],
                                         scale=1.0)
                else:
                    nc.vector.tensor_scalar_add(out, in_, bt[:])

            # --- main loops: class-major; image 0 tap-major.  The
            # final class uses [15,15,3] chunks so only a 3-row drain
            # and a 45KB DMA trail the last matmul. -------------------
            out_engs = [nc.sync, nc.gpsimd, nc.scalar]
            oi = 0
            for b in range(B_PER):
                for ci_, (p, q) in enumerate(CLASSES):
                    last = (b == B_PER - 1) and (p, q) == CLASSES[-1]
                    ct = cpool.tile([128, 33, NW], BF16)
                    if b == 0:
                        pss = [ppool.tile([128, NF], F32, name="ps")
                               for _ in range(NCH)]
                        _emit_class_tapmajor(nc, pss, wt, xv[b], p, q)
                        for r in range(NCH):
                            drain(r, ct, pss[r], R * r, R)
                    else:
                        chunks = [(0, 15), (15, 15), (30, 2), (32, 1)] \
                            if last else [(0, R), (R, R), (2 * R, R)]
                        for k, (row0, rows) in enumerate(chunks):
                            ps = ppool.tile([128, rows * NW], F32,
                                            name="ps")
                            _emit_group(nc, ps, wt, xv[b], p, q,
                                        row0, rows)
                            drain(k, ct, ps, row0, rows)
                            if last:
                                eng = out_engs[(oi + k) % 3]
                                eng.dma_start(
                                    out=y_d[b, p, q, :, row0:row0 + rows],
                                    in_=ct[:, row0:row0 + rows])
                    if not last:
                        eng = out_engs[oi % 3]
                        oi += 1
                        eng.dma_start(out=y_d[b, p, q], in_=ct[:])
                    if b == 0 and ci_ == 0:
                        # WAW gate: a 1-column write into the image-1 x
                        # tiles, dependent on the first class-00 drain,
                        # holds the 256-packet x1 DMAs out of the
                        # startup window (the scheduler hoists any
                        # dependency-free DMA onto an idle engine).
                        nc.scalar.activation(xp[1][0][:, 0:1],
                                             ct[:, 0, 0:1], COPY)
                        nc.scalar.activation(xp[1][1][:, 0:1],
                                             ct[:, 0, 0:1], COPY)
                        nc.gpsimd.dma_start(out=xp[1][0][:],
                                            in_=x_d[1, 0:128])
                        nc.gpsimd.dma_start(out=xp[1][1][:],
                                            in_=x_d[1, 128:256])

    nc.compile()
    return nc


_nc_cache = None


def _get_nc():
    global _nc_cache
    if _nc_cache is None:
        _nc_cache = build_nc()
    return _nc_cache


def make_in_maps(x: np.ndarray, weight: np.ndarray, bias: np.ndarray):
    # w[ci,co,kh,kw] -> [c, ci', class(2p+q), (2i+j)*co], bf16
    w6 = (
        np.asarray(weight, dtype=np.float32)
        .reshape(2, 128, 128, 2, 2, 2, 2)      # [c, ci', co, i, p, j, q]
        .transpose(0, 1, 4, 6, 3, 5, 2)        # -> [c, ci', p, q, i, j, co]
        .reshape(2, 128, 4, 512)
    )
    w_host = np.ascontiguousarray(w6.astype(ml_dtypes.bfloat16))
    b_host = np.ascontiguousarray(
        np.asarray(bias, dtype=np.float32).reshape(128, 1)
    )
    x = np.asarray(x, dtype=np.float32)
    # host-side zero-pad into the 34x34(+tail) bf16 layout the kernel reads
    xpad = np.zeros((16, 256, XPAD), dtype=ml_dtypes.bfloat16)
    xpad[:, :, :XLEN].reshape(16, 256, PW, PW)[:, :, 1:33, 1:33] = \
        x.astype(ml_dtypes.bfloat16)
    return [
        {
            "x": np.ascontiguousarray(xpad[B_PER * i:B_PER * (i + 1)]),
            "w": w_host,
            "b": b_host,
        }
        for i in range(N_CORES)
    ]


def kernel(x: np.ndarray, weight: np.ndarray, bias: np.ndarray) -> np.ndarray:
    nc = _get_nc()
    in_maps = make_in_maps(x, weight, bias)
    res = run_bass_kernel_spmd(nc, in_maps, list(range(N_CORES)))
    out = np.empty((16, 128, 66, 66), dtype=np.float32)
    for i, r in enumerate(res.results):
        y = np.asarray(r["y"]).reshape(B_PER, 2, 2, 128, NCH * R, NW)
        for b in range(B_PER):
            for p in range(2):
                for q in range(2):
                    out[B_PER * i + b, :, p::2, q::2] = \
                        y[b, p, q].astype(np.float32)
    return out


# revision 42
# speedup vs baseline: 1.1231x; 1.1231x over previous
"""ConvTranspose2d (16,256,32,32) -> (16,128,66,66), stride 2, 4x4 kernel.

Strategy: data-parallel over batch, 2 images per core on 8 NeuronCores.

Math: y[b,co,2m+p,2n+q] = bias[co]
        + sum_{i,j in {0,1}} sum_ci x[b,ci,m-i,n-j] * w[ci,co,p+2i,q+2j]
for parity class (p,q) in {0,1}^2, m,n in [0,33).

All-bf16 datapath (x, w, y; fp32 PSUM/bias): the 2e-2 rel-err gate
leaves huge headroom (measured ~3e-3), halves DMA, and avoids the
fp32-HIGH PE power mode.

Per image and parity class: output subgrid [128co x 33 x 33] as 3
row-chunks of 11 rows; each chunk is one PSUM group of 8 matmuls
(2 ci-chunks x 4 taps), K=128, M=128co, N=363 via a 2D strided rhs
over the zero-padded 34x34 SBUF x copy (padded host-side).  Drains
fuse the bias add, write contiguous bf16 class tiles; host does the
parity de-interleave.  Output leaves per class; the last class leaves
as 3 per-chunk DMAs so only ~90KB trails the final matmul.

Timing model (from traces): every [128-part, *] DMA is >=128 packets
served by the 16 shared DMA engines, so the first matmul is gated by
total packets in flight in the startup window.  Wave 1 is only
w[c0,class00] (1KB rows) + x image0 c0.  Image-0 runs tap-major
(c-outer), pushing the x0c1/w-rest deadlines to matmul 13/25.  The
image-1 x loads are held back by a WAW gate (a 1-column write into
their tiles dependent on the first class-00 drain) so their 256
packets stay out of the startup window.  The PE clock ramps until
~7us of accumulated busy time (warmups bridge the DMA window), HAM
then grants full speed for a fixed ~31-34us window, and a fixed
~257-instruction semaphore-clear teardown (~3-6us) follows the last
output DMA -- the schedule squeezes most work inside the full-speed
window.
"""

import numpy as np
import ml_dtypes

import concourse.bass as bass
import concourse.bacc as bacc
import concourse.tile as tile
from concourse import mybir
from concourse.bass_utils import run_bass_kernel_spmd

N_CORES = 8
B_PER = 2  # images per core

F32 = mybir.dt.float32
BF16 = mybir.dt.bfloat16

PW = 34            # padded x width (32 + 1 left + 1 right)
XLEN = PW * PW     # 1156 padded x elems per partition
XPAD = 1160        # sbuf/dram x free size (tail slack, keeps 4B align)
R = 11             # output parity rows per PSUM chunk
NCH = 3            # chunks: 3 * 11 = 33 parity rows
NW = 33            # useful output cols per parity row
NF = R * NW        # 363 matmul free dim (2D strided rhs AP)
NWARM = 7          # PE clock-ramp warmup matmuls: fill the PE-idle DMA
                   # window (~7.6-10.4us) so HAM's accumulated-busy
                   # full-speed grant comes earlier

CLASSES = [(0, 0), (0, 1), (1, 0), (1, 1)]
COPY = mybir.ActivationFunctionType.Copy
IDENT = mybir.ActivationFunctionType.Identity


def _mm(nc, ps, wt, xv, p, q, c, i, j, row0, rows, start, stop):
    r0 = row0 - i + 1
    c0 = 1 - j
    nc.tensor.matmul(
        ps[:],
        wt[c][:, 2 * p + q, (2 * i + j) * 128:(2 * i + j + 1) * 128],
        xv[c][:, r0:r0 + rows, c0:c0 + NW],
        start=start,
        stop=stop,
        skip_group_check=True,
    )


def _emit_class_tapmajor(nc, pss, wt, xv, p, q):
    """Taps outer (c0 first: x0c1 not needed until matmul 13), chunks
    inner; the 3 PSUM groups accumulate interleaved."""
    k = 0
    for c in range(2):
        for i in range(2):
            for j in range(2):
                for r in range(NCH):
                    _mm(nc, pss[r], wt, xv, p, q, c, i, j, R * r, R,
                        start=(k == 0), stop=(k == 7))
                k += 1


def _emit_group(nc, ps, wt, xv, p, q, row0, rows):
    """Chunk-major: one PSUM group of 8 matmuls (early drains)."""
    k = 0
    for c in range(2):
        for i in range(2):
            for j in range(2):
                _mm(nc, ps, wt, xv, p, q, c, i, j, row0, rows,
                    start=(k == 0), stop=(k == 7))
                k += 1


def build_nc(debug: bool = False) -> bass.Bass:
    nc = bacc.Bacc("TRN2", target_bir_lowering=False, debug=debug,
                   num_devices=N_CORES)

    # x arrives host-padded bf16: 34x34 zero-border layout + tail pad
    x_d = nc.declare_dram_parameter("x", [B_PER, 256, XPAD], BF16,
                                    isOutput=False)
    # w layout: [ci_chunk, ci, class(2p+q), tap(2i+j)*co] -- contiguous
    # per-(c,class) DRAM runs
    w_d = nc.declare_dram_parameter("w", [2, 128, 4, 512], BF16,
                                    isOutput=False)
    b_d = nc.declare_dram_parameter("b", [128, 1], F32, isOutput=False)
    # class-major output: host de-interleaves parity grids
    y_d = nc.declare_dram_parameter("y", [B_PER, 2, 2, 128, 33, NW],
                                    BF16, isOutput=True)

    with tile.TileContext(nc) as tc:
        with (
            tc.tile_pool(name="wp", bufs=2) as wpool,
            tc.tile_pool(name="bp", bufs=1) as bpool,
            tc.tile_pool(name="xp", bufs=2 * B_PER) as xpool,
            tc.tile_pool(name="cp", bufs=3) as cpool,
            tc.tile_pool(name="ps", bufs=6, space="PSUM") as ppool,
            tc.tile_pool(name="pw", bufs=1, space="PSUM") as warmpool,
        ):
            # --- tiles ----------------------------------------------
            wub = bpool.tile([128, 512], BF16)
            bt = bpool.tile([128, 1], F32)
            wt = [wpool.tile([128, 4, 512], BF16, name=f"wt{c}", tag="wt")
                  for c in range(2)]
            xp = [[xpool.tile([128, XPAD], BF16, name=f"x{b}c{c}", tag="xt")
                   for c in range(2)] for b in range(B_PER)]
            xv = [[xp[b][c][:, 0:XLEN].rearrange("p (h w) -> p h w", w=PW)
                   for c in range(2)] for b in range(B_PER)]

            # --- warmup ---------------------------------------------
            # gpsimd's program loads first: the wub memset goes there
            # so the PE can start at ~7.5us.
            nc.gpsimd.memset(wub[:], 0.0)
            wps = warmpool.tile([128, 512], F32)
            for _ in range(NWARM):
                nc.tensor.matmul(wps[:], wub[:, 0:128], wub[:],
                                 start=True, stop=True)

            # --- input DMA waves ------------------------------------
            # wave 1 (priority 0): just the two first-matmul gates
            # (256 packets).  Wave 2: ALL on scalar's queue in strict
            # consumption order — the ~650ns per-issue cost staggers
            # each ring's service start without completion barriers
            # (hard WAW serialization measured +5us; concurrent
            # round-robin with deadline-ordered starts wins).
            # Deadlines: x0c1 + w1[class00] at matmul 13, w0-rest at
            # 25, w1-rest at 37, bias at the first drain (which has
            # PSUM-rotation slack).
            with tc.high_priority():
                nc.sync.dma_start(out=wt[0][:, 0], in_=w_d[0, :, 0])
                nc.scalar.dma_start(out=xp[0][0][:], in_=x_d[0, 0:128])
            nc.scalar.dma_start(out=xp[0][1][:], in_=x_d[0, 128:256])
            nc.scalar.dma_start(out=wt[1][:, 0], in_=w_d[1, :, 0])
            nc.scalar.dma_start(out=wt[0][:, 1:4], in_=w_d[0, :, 1:4])
            nc.scalar.dma_start(out=wt[1][:, 1:4], in_=w_d[1, :, 1:4])
            nc.scalar.dma_start(out=bt[:], in_=b_d[:])

            # --- drains: alternate vector/scalar (gpsimd cannot read
            # PSUM) ---------------------------------------------------
            def drain(k, ct, ps, row0, rows):
                in_ = ps[:].rearrange("p (m n) -> p m n", n=NW)
                out = ct[:, row0:row0 + rows]
                if k == 1:
                    nc.scalar.activation(out, in_, IDENT, bias=bt[:],
                                         scale=1.0)
                else:
                    nc.vector.tensor_scalar_add(out, in_, bt[:])

            # --- main loops: class-major; image 0 tap-major.  The
            # final class uses [15,15,3] chunks so only a 3-row drain
            # and a 45KB DMA trail the last matmul. -------------------
            out_engs = [nc.sync, nc.gpsimd, nc.scalar]
            oi = 0
            for b in range(B_PER):
                for ci_, (p, q) in enumerate(CLASSES):
                    last = (b == B_PER - 1) and (p, q) == CLASSES[-1]
                    ct = cpool.tile([128, 33, NW], BF16)
                    if b == 0:
                        pss = [ppool.tile([128, NF], F32, name="ps")
                               for _ in range(NCH)]
                        _emit_class_tapmajor(nc, pss, wt, xv[b], p, q)
                        for r in range(NCH):
                            drain(r, ct, pss[r], R * r, R)
                    else:
                        chunks = [(0, 15), (15, 15), (30, 2), (32, 1)] \
                            if last else [(0, R), (R, R), (2 * R, R)]
                        for k, (row0, rows) in enumerate(chunks):
                            ps = ppool.tile([128, rows * NW], F32,
                                            name="ps")
                            _emit_group(nc, ps, wt, xv[b], p, q,
                                        row0, rows)
                            drain(k, ct, ps, row0, rows)
                            if last:
                                eng = out_engs[(oi + k) % 3]
                                eng.dma_start(
                                    out=y_d[b, p, q, :, row0:row0 + rows],
                                    in_=ct[:, row0:row0 + rows])
                    if not last:
                        eng = out_engs[oi % 3]
                        oi += 1
                        eng.dma_start(out=y_d[b, p, q], in_=ct[:])
                    if b == 0 and ci_ == 0:
                        # WAW gate: a 1-column write into the image-1 x
                        # tiles, dependent on the first class-00 drain,
                        # holds the 256-packet x1 DMAs out of the
                        # startup window (the scheduler hoists any
                        # dependency-free DMA onto an idle engine).
                        nc.scalar.activation(xp[1][0][:, 0:1],
                                             ct[:, 0, 0:1], COPY)
                        nc.scalar.activation(xp[1][1][:, 0:1],
                                             ct[:, 0, 0:1], COPY)
                        nc.gpsimd.dma_start(out=xp[1][0][:],
                                            in_=x_d[1, 0:128])
                        nc.gpsimd.dma_start(out=xp[1][1][:],
                                            in_=x_d[1, 128:256])

    nc.compile()
    return nc


_nc_cache = None


def _get_nc():
    global _nc_cache
    if _nc_cache is None:
        _nc_cache = build_nc()
    return _nc_cache


def make_in_maps(x: np.ndarray, weight: np.ndarray, bias: np.ndarray):
    # w[ci,co,kh,kw] -> [c, ci', class(2p+q), (2i+j)*co], bf16
    w6 = (
        np.asarray(weight, dtype=np.float32)
        .reshape(2, 128, 128, 2, 2, 2, 2)      # [c, ci', co, i, p, j, q]
        .transpose(0, 1, 4, 6, 3, 5, 2)        # -> [c, ci', p, q, i, j, co]
        .reshape(2, 128, 4, 512)
    )
    w_host = np.ascontiguousarray(w6.astype(ml_dtypes.bfloat16))
    b_host = np.ascontiguousarray(
        np.asarray(bias, dtype=np.float32).reshape(128, 1)
    )
    x = np.asarray(x, dtype=np.float32)
    # host-side zero-pad into the 34x34(+tail) bf16 layout the kernel reads
    xpad = np.zeros((16, 256, XPAD), dtype=ml_dtypes.bfloat16)
    xpad[:, :, :XLEN].reshape(16, 256, PW, PW)[:, :, 1:33, 1:33] = \
        x.astype(ml_dtypes.bfloat16)
    return [
        {
            "x": np.ascontiguousarray(xpad[B_PER * i:B_PER * (i + 1)]),
            "w": w_host,
            "b": b_host,
        }
        for i in range(N_CORES)
    ]


def kernel(x: np.ndarray, weight: np.ndarray, bias: np.ndarray) -> np.ndarray:
    nc = _get_nc()
    in_maps = make_in_maps(x, weight, bias)
    res = run_bass_kernel_spmd(nc, in_maps, list(range(N_CORES)))
    out = np.empty((16, 128, 66, 66), dtype=np.float32)
    for i, r in enumerate(res.results):
        y = np.asarray(r["y"]).reshape(B_PER, 2, 2, 128, NCH * R, NW)
        for b in range(B_PER):
            for p in range(2):
                for q in range(2):
                    out[B_PER * i + b, :, p::2, q::2] = \
                        y[b, p, q].astype(np.float32)
    return out


# revision 44
# speedup vs baseline: 1.1271x; 1.0035x over previous
"""ConvTranspose2d (16,256,32,32) -> (16,128,66,66), stride 2, 4x4 kernel.

Strategy: data-parallel over batch, 2 images per core on 8 NeuronCores.

Math: y[b,co,2m+p,2n+q] = bias[co]
        + sum_{i,j in {0,1}} sum_ci x[b,ci,m-i,n-j] * w[ci,co,p+2i,q+2j]
for parity class (p,q) in {0,1}^2, m,n in [0,33).

All-bf16 datapath (x, w, y; fp32 PSUM/bias): the 2e-2 rel-err gate
leaves huge headroom (measured ~3e-3), halves DMA, and avoids the
fp32-HIGH PE power mode.

Per image and parity class: output subgrid [128co x 33 x 33] as 3
row-chunks of 11 rows; each chunk is one PSUM group of 8 matmuls
(2 ci-chunks x 4 taps), K=128, M=128co, N=363 via a 2D strided rhs
over the zero-padded 34x34 SBUF x copy (padded host-side).  Drains
fuse the bias add, write contiguous bf16 class tiles; host does the
parity de-interleave.  Output leaves per class; the last class leaves
as 3 per-chunk DMAs so only ~90KB trails the final matmul.

Timing model (from traces): every [128-part, *] DMA is >=128 packets
served by the 16 shared DMA engines, so the first matmul is gated by
total packets in flight in the startup window.  Wave 1 is only
w[c0,class00] (1KB rows) + x image0 c0.  Image-0 runs tap-major
(c-outer), pushing the x0c1/w-rest deadlines to matmul 13/25.  The
image-1 x loads are held back by a WAW gate (a 1-column write into
their tiles dependent on the first class-00 drain) so their 256
packets stay out of the startup window.  The PE clock ramps until
~7us of accumulated busy time (warmups bridge the DMA window), HAM
then grants full speed for a fixed ~31-34us window, and a fixed
~257-instruction semaphore-clear teardown (~3-6us) follows the last
output DMA -- the schedule squeezes most work inside the full-speed
window.
"""

import numpy as np
import ml_dtypes

import concourse.bass as bass
import concourse.bacc as bacc
import concourse.tile as tile
from concourse import mybir
from concourse.bass_utils import run_bass_kernel_spmd

N_CORES = 8
B_PER = 2  # images per core

F32 = mybir.dt.float32
BF16 = mybir.dt.bfloat16

PW = 34            # padded x width (32 + 1 left + 1 right)
XLEN = PW * PW     # 1156 padded x elems per partition
XPAD = 1160        # sbuf/dram x free size (tail slack, keeps 4B align)
R = 11             # output parity rows per PSUM chunk
NCH = 3            # chunks: 3 * 11 = 33 parity rows
NW = 33            # useful output cols per parity row
NF = R * NW        # 363 matmul free dim (2D strided rhs AP)
NWARM = 7          # PE clock-ramp warmup matmuls: fill the PE-idle DMA
                   # window (~7.6-10.4us) so HAM's accumulated-busy
                   # full-speed grant comes earlier

CLASSES = [(0, 0), (0, 1), (1, 0), (1, 1)]
COPY = mybir.ActivationFunctionType.Copy
IDENT = mybir.ActivationFunctionType.Identity


def _mm(nc, ps, wt, xv, p, q, c, i, j, row0, rows, start, stop):
    r0 = row0 - i + 1
    c0 = 1 - j
    nc.tensor.matmul(
        ps[:],
        wt[c][:, 2 * p + q, (2 * i + j) * 128:(2 * i + j + 1) * 128],
        xv[c][:, r0:r0 + rows, c0:c0 + NW],
        start=start,
        stop=stop,
        skip_group_check=True,
    )


def _emit_class_tapmajor(nc, pss, wt, xv, p, q):
    """Taps outer (c0 first: x0c1 not needed until matmul 13), chunks
    inner; the 3 PSUM groups accumulate interleaved."""
    k = 0
    for c in range(2):
        for i in range(2):
            for j in range(2):
                for r in range(NCH):
                    _mm(nc, pss[r], wt, xv, p, q, c, i, j, R * r, R,
                        start=(k == 0), stop=(k == 7))
                k += 1


def _emit_group(nc, ps, wt, xv, p, q, row0, rows):
    """Chunk-major: one PSUM group of 8 matmuls (early drains)."""
    k = 0
    for c in range(2):
        for i in range(2):
            for j in range(2):
                _mm(nc, ps, wt, xv, p, q, c, i, j, row0, rows,
                    start=(k == 0), stop=(k == 7))
                k += 1


def build_nc(debug: bool = False) -> bass.Bass:
    nc = bacc.Bacc("TRN2", target_bir_lowering=False, debug=debug,
                   num_devices=N_CORES)

    # x arrives host-padded bf16: 34x34 zero-border layout + tail pad
    x_d = nc.declare_dram_parameter("x", [B_PER, 256, XPAD], BF16,
                                    isOutput=False)
    # w layout: [ci_chunk, ci, class(2p+q), tap(2i+j)*co] -- contiguous
    # per-(c,class) DRAM runs
    w_d = nc.declare_dram_parameter("w", [2, 128, 4, 512], BF16,
                                    isOutput=False)
    b_d = nc.declare_dram_parameter("b", [128, 1], F32, isOutput=False)
    # class-major output: host de-interleaves parity grids
    y_d = nc.declare_dram_parameter("y", [B_PER, 2, 2, 128, 33, NW],
                                    BF16, isOutput=True)

    with tile.TileContext(nc) as tc:
        with (
            tc.tile_pool(name="wp", bufs=2) as wpool,
            tc.tile_pool(name="bp", bufs=1) as bpool,
            tc.tile_pool(name="xp", bufs=2 * B_PER) as xpool,
            tc.tile_pool(name="cp", bufs=3) as cpool,
            tc.tile_pool(name="ps", bufs=6, space="PSUM") as ppool,
            tc.tile_pool(name="pw", bufs=1, space="PSUM") as warmpool,
        ):
            # --- tiles ----------------------------------------------
            wub = bpool.tile([128, 512], BF16)
            dum = bpool.tile([128, 512], BF16)
            bt = bpool.tile([128, 1], F32)
            wt = [wpool.tile([128, 4, 512], BF16, name=f"wt{c}", tag="wt")
                  for c in range(2)]
            xp = [[xpool.tile([128, XPAD], BF16, name=f"x{b}c{c}", tag="xt")
                   for c in range(2)] for b in range(B_PER)]
            xv = [[xp[b][c][:, 0:XLEN].rearrange("p (h w) -> p h w", w=PW)
                   for c in range(2)] for b in range(B_PER)]

            # --- warmup ---------------------------------------------
            # gpsimd's program loads first: the wub memset goes there
            # so the PE can start at ~7.5us.
            nc.gpsimd.memset(wub[:], 0.0)
            wps = warmpool.tile([128, 512], F32)
            for _ in range(NWARM):
                nc.tensor.matmul(wps[:], wub[:, 0:128], wub[:],
                                 start=True, stop=True)

            # --- input DMA waves ------------------------------------
            # wave 1 (priority 0): just the two first-matmul gates
            # (256 packets).  Wave 2: ALL on scalar's queue in strict
            # consumption order — the ~650ns per-issue cost staggers
            # each ring's service start without completion barriers
            # (hard WAW serialization measured +5us; concurrent
            # round-robin with deadline-ordered starts wins).
            # Deadlines: x0c1 + w1[class00] at matmul 13, w0-rest at
            # 25, w1-rest at 37, bias at the first drain (which has
            # PSUM-rotation slack).
            with tc.high_priority():
                nc.sync.dma_start(out=wt[0][:, 0], in_=w_d[0, :, 0])
                nc.scalar.dma_start(out=xp[0][0][:], in_=x_d[0, 0:128])
            nc.scalar.dma_start(out=xp[0][1][:], in_=x_d[0, 128:256])
            nc.scalar.dma_start(out=wt[1][:, 0], in_=w_d[1, :, 0])
            # ~1.2us of dummy scalar work widens the stagger between
            # the matmul-13-critical rings above and the bulk below,
            # without a completion barrier
            nc.scalar.activation(dum[:], wub[:], COPY)
            nc.scalar.activation(dum[:], wub[:], COPY)
            nc.scalar.dma_start(out=wt[0][:, 1:4], in_=w_d[0, :, 1:4])
            nc.scalar.dma_start(out=wt[1][:, 1:4], in_=w_d[1, :, 1:4])
            nc.scalar.dma_start(out=bt[:], in_=b_d[:])

            # --- drains: alternate vector/scalar (gpsimd cannot read
            # PSUM) ---------------------------------------------------
            def drain(k, ct, ps, row0, rows):
                in_ = ps[:].rearrange("p (m n) -> p m n", n=NW)
                out = ct[:, row0:row0 + rows]
                if k == 1:
                    nc.scalar.activation(out, in_, IDENT, bias=bt[:],
                                         scale=1.0)
                else:
                    nc.vector.tensor_scalar_add(out, in_, bt[:])

            # --- main loops: class-major; image 0 tap-major.  The
            # final class uses [15,15,3] chunks so only a 3-row drain
            # and a 45KB DMA trail the last matmul. -------------------
            out_engs = [nc.sync, nc.gpsimd, nc.scalar]
            oi = 0
            for b in range(B_PER):
                for ci_, (p, q) in enumerate(CLASSES):
                    last = (b == B_PER - 1) and (p, q) == CLASSES[-1]
                    ct = cpool.tile([128, 33, NW], BF16)
                    if b == 0:
                        pss = [ppool.tile([128, NF], F32, name="ps")
                               for _ in range(NCH)]
                        _emit_class_tapmajor(nc, pss, wt, xv[b], p, q)
                        for r in range(NCH):
                            drain(r, ct, pss[r], R * r, R)
                    else:
                        chunks = [(0, 15), (15, 15), (30, 2), (32, 1)] \
                            if last else [(0, R), (R, R), (2 * R, R)]
                        for k, (row0, rows) in enumerate(chunks):
                            ps = ppool.tile([128, rows * NW], F32,
                                            name="ps")
                            _emit_group(nc, ps, wt, xv[b], p, q,
                                        row0, rows)
                            drain(k, ct, ps, row0, rows)
                            if last:
                                eng = out_engs[(oi + k) % 3]
                                eng.dma_start(
                                    out=y_d[b, p, q, :, row0:row0 + rows],
                                    in_=ct[:, row0:row0 + rows])
                    if not last:
                        eng = out_engs[oi % 3]
                        oi += 1
                        eng.dma_start(out=y_d[b, p, q], in_=ct[:])
                    if b == 0 and ci_ == 0:
                        # WAW gate: a 1-column write into the image-1 x
                        # tiles, dependent on the first class-00 drain,
                        # holds the 256-packet x1 DMAs out of the
                        # startup window (the scheduler hoists any
                        # dependency-free DMA onto an idle engine).
                        nc.scalar.activation(xp[1][0][:, 0:1],
                                             ct[:, 0, 0:1], COPY)
                        nc.scalar.activation(xp[1][1][:, 0:1],
                                             ct[:, 0, 0:1], COPY)
                        nc.gpsimd.dma_start(out=xp[1][0][:],
                                            in_=x_d[1, 0:128])
                        nc.gpsimd.dma_start(out=xp[1][1][:],
                                            in_=x_d[1, 128:256])

    nc.compile()
    return nc


_nc_cache = None


def _get_nc():
    global _nc_cache
    if _nc_cache is None:
        _nc_cache = build_nc()
    return _nc_cache


def make_in_maps(x: np.ndarray, weight: np.ndarray, bias: np.ndarray):
    # w[ci,co,kh,kw] -> [c, ci', class(2p+q), (2i+j)*co], bf16
    w6 = (
        np.asarray(weight, dtype=np.float32)
        .reshape(2, 128, 128, 2, 2, 2, 2)      # [c, ci', co, i, p, j, q]
        .transpose(0, 1, 4, 6, 3, 5, 2)        # -> [c, ci', p, q, i, j, co]
        .reshape(2, 128, 4, 512)
    )
    w_host = np.ascontiguousarray(w6.astype(ml_dtypes.bfloat16))
    b_host = np.ascontiguousarray(
        np.asarray(bias, dtype=np.float32).reshape(128, 1)
    )
    x = np.asarray(x, dtype=np.float32)
    # host-side zero-pad into the 34x34(+tail) bf16 layout the kernel reads
    xpad = np.zeros((16, 256, XPAD), dtype=ml_dtypes.bfloat16)
    xpad[:, :, :XLEN].reshape(16, 256, PW, PW)[:, :, 1:33, 1:33] = \
        x.astype(ml_dtypes.bfloat16)
    return [
        {
            "x": np.ascontiguousarray(xpad[B_PER * i:B_PER * (i + 1)]),
            "w": w_host,
            "b": b_host,
        }
        for i in range(N_CORES)
    ]


def kernel(x: np.ndarray, weight: np.ndarray, bias: np.ndarray) -> np.ndarray:
    nc = _get_nc()
    in_maps = make_in_maps(x, weight, bias)
    res = run_bass_kernel_spmd(nc, in_maps, list(range(N_CORES)))
    out = np.empty((16, 128, 66, 66), dtype=np.float32)
    for i, r in enumerate(res.results):
        y = np.asarray(r["y"]).reshape(B_PER, 2, 2, 128, NCH * R, NW)
        for b in range(B_PER):
            for p in range(2):
                for q in range(2):
                    out[B_PER * i + b, :, p::2, q::2] = \
                        y[b, p, q].astype(np.float32)
    return out
